# revision 4
# baseline (speedup 1.0000x reference)
"""CrossAttention2D Trainium2 Bass kernel.

Problem (per batch item b, C=128, HW=64*64=4096):
    q = Wq @ xq + bq            # [C, HW]   (1x1 conv == GEMM)
    k = Wk @ xk + bk            # [C, HW]
    S = (q^T k) / sqrt(HW)      # [HW, HW]
    A = softmax(S, axis=-1)
    out = (A @ v^T)^T + q       # [C, HW],  v = xv
Sharding: data-parallel over batch B=8 -> one batch item per NeuronCore.

Per-core pipeline (no collectives):
  - Q/K projections in bf16 (inputs+weights cast; PSUM f32; Q's f32 result
    feeds the residual, which dominates the output norm -> bf16-level noise
    on the attention path is ~64x diluted and well inside the 2e-2 gate).
  - Scores computed TRANSPOSED: S^T tiles [tk=128, tq=1024], bf16 matmuls.
  - exp on ScalarE -> fp8 e4m3 tiles written in the DoubleRowSwInterleave
    weight layout (tk-block pairs interleaved per column, columns reversed)
    so the PV matmuls can run in fp8 DoubleRow-SwInterleave mode: one
    matmul per (tk-pair, tq-block) contracts 256 tks -> half the PV ops.
  - V transposed on the PE to vt[tk, c] fp8 with a ones column (col 128)
    so PV accumulates the softmax denominator for free.
  - Finalize (software-pipelined into the next chunk): DVE normalize,
    PE transpose back to [c, tq], DVE residual add, DMA out.
"""

import os
import numpy as np

B, C, H, W = 8, 128, 64, 64
HW = H * W            # 4096
P = 128
TQ = 512              # moving free dim of one S^T matmul (PSUM bank width)
TQC = 1024            # query-token chunk (2 banks -> one FD=1024 exp)
NCHUNK = HW // TQC    # 4
NTK = HW // P         # 32 key blocks
ND = NTK // 2         # 16 double-blocks (DoubleRow pairs)
VT_STRIDE = 130       # 129 used + 1 pad
PREFD = 3             # exp D-pairs emitted before the previous finalize
OPACK = 3             # accumulator groups packed per PSUM bank

_CACHE: dict = {}
LAST_RESULTS = None   # BassKernelResults of the most recent run (for test.py)


def _build_kernel():
    import concourse.tile as tile
    from concourse import bacc, mybir
    from concourse.masks import make_identity

    f32 = mybir.dt.float32
    bf16 = mybir.dt.bfloat16
    fp8 = mybir.dt.float8e4
    AF = mybir.ActivationFunctionType
    DRSW = mybir.MatmulPerfMode.DoubleRowSwInterleave

    nc = bacc.Bacc("TRN2", target_bir_lowering=False, debug=False)

    xq = nc.dram_tensor("xq", [C, HW], f32, kind="ExternalInput")
    xk = nc.dram_tensor("xk", [C, HW], f32, kind="ExternalInput")
    xv = nc.dram_tensor("xv", [C, HW], f32, kind="ExternalInput")
    wqT = nc.dram_tensor("wqT", [C, C], f32, kind="ExternalInput")
    wkT = nc.dram_tensor("wkT", [C, C], f32, kind="ExternalInput")
    bqv = nc.dram_tensor("bqv", [C, 1], f32, kind="ExternalInput")
    bkv = nc.dram_tensor("bkv", [C, 1], f32, kind="ExternalInput")
    out = nc.dram_tensor("out", [C, HW], f32, kind="ExternalOutput")

    inv_sqrt_hw = 1.0 / float(np.sqrt(HW))
    i8 = mybir.dt.int8
    SCH_A = 8.0 * float(np.log2(np.e)) * inv_sqrt_hw
    SCH_B = 56.0 - 0.344

    with tile.TileContext(nc) as tc:
        with (
            tc.tile_pool(name="const", bufs=1) as cpool,
            tc.tile_pool(name="stage", bufs=1) as spool,
            tc.tile_pool(name="expp", bufs=6) as epool,
            tc.tile_pool(name="fin", bufs=3) as fpool,
            tc.tile_pool(name="ps_s", bufs=2, space="PSUM") as pss,
        ):
            # ---------- constants / weights ----------
            wq_sb = cpool.tile([C, C], f32, name="wq_sb")
            wk_sb = cpool.tile([C, C], f32, name="wk_sb")
            wq_bf = cpool.tile([C, C], bf16, name="wq_bf")
            wk_bf = cpool.tile([C, C], bf16, name="wk_bf")
            bq_sb = cpool.tile([C, 1], f32, name="bq_sb")
            bk_sb = cpool.tile([C, 1], f32, name="bk_sb")
            ident_b = cpool.tile([P, P], bf16, name="ident_b")
            ident_f = cpool.tile([P, P], f32, name="ident_f")
            nc.sync.dma_start(wq_sb[:], wqT[:])
            nc.sync.dma_start(wk_sb[:], wkT[:])
            nc.sync.dma_start(bq_sb[:], bqv[:])
            nc.sync.dma_start(bk_sb[:], bkv[:])
            nc.vector.tensor_copy(wq_bf[:], wq_sb[:])
            nc.vector.tensor_copy(wk_bf[:], wk_sb[:])
            make_identity(nc, ident_b)
            make_identity(nc, ident_f)

            # ---------- input staging + bf16 casts ----------
            xq_sb = spool.tile([C, HW], f32, name="xq_sb")
            xk_sb = spool.tile([C, HW], f32, name="xk_sb")
            xv_sb = spool.tile([C, HW], f32, name="xv_sb")
            xq_bf = spool.tile([C, HW], bf16, name="xq_bf")
            xk_bf = spool.tile([C, HW], bf16, name="xk_bf")

            def stage(dst, src, j, cast=None):
                sl = slice(j * TQ, (j + 1) * TQ)
                nc.sync.dma_start(dst[:, sl], src[:, sl])
                if cast is not None:
                    nc.gpsimd.tensor_copy(cast[:, sl], dst[:, sl])

            for j in range(TQC // TQ):
                stage(xq_sb, xq, j, xq_bf)
            stage(xk_sb, xk, 0, xk_bf)
            stage(xk_sb, xk, 1, xk_bf)
            for j in range(HW // TQ):
                stage(xv_sb, xv, j)
            for j in range(2, HW // TQ):
                stage(xk_sb, xk, j, xk_bf)
            for j in range(TQC // TQ, HW // TQ):
                stage(xq_sb, xq, j, xq_bf)

            # ---------- projections (bias add + PSUM evac on DVE) ----------
            q_f32 = spool.tile([C, HW], f32, name="q_f32")
            q_bf = spool.tile([C, HW], bf16, name="q_bf")
            k_bf = spool.tile([C, HW], bf16, name="k_bf")

            pst = tc.alloc_tile_pool(name="ps_t", bufs=1, space="PSUM")

            def q_proj(j, pool, tag):
                sl = slice(j * TQ, (j + 1) * TQ)
                qp = pool.tile([P, TQ], f32, name="qp", tag=tag)
                nc.tensor.matmul(qp[:], wq_bf[:], xq_bf[:, sl],
                                 start=True, stop=True)
                nc.vector.tensor_scalar_add(q_f32[:, sl], qp[:], bq_sb[:])
                nc.vector.tensor_copy(q_bf[:, sl], q_f32[:, sl])

            def k_proj(j, pool, tag):
                sl = slice(j * TQ, (j + 1) * TQ)
                kp = pool.tile([P, TQ], f32, name="kp", tag=tag)
                nc.tensor.matmul(kp[:], wk_bf[:], xk_bf[:, sl],
                                 start=True, stop=True)
                nc.vector.tensor_scalar_add(k_bf[:, sl], kp[:], bk_sb[:])

            # PE warm-up: a few dependency-light matmuls ramp the tensor
            # engine clock while the input DMAs stream; dedicated tag so the
            # projection psum ring is not WAR-chained behind them.
            wz = cpool.tile([P, TQ], bf16, name="wz")
            nc.gpsimd.memset(wz[:], 0.0)
            for w in range(4):
                wrm = pst.tile([P, TQ], f32, name="wrm", tag="t")
                nc.tensor.matmul(wrm[:], ident_b[:], wz[:],
                                 start=True, stop=True, skip_group_check=True)
            q_proj(0, pss, "ps")
            q_proj(1, pss, "ps")
            k_proj(0, pss, "ps")
            k_proj(1, pss, "ps")
            k_done = 2

            TQ4 = 256

            def q_proj256(u):
                sl = slice(u * TQ4, (u + 1) * TQ4)
                qp = pst.tile([P, TQ4], f32, name="qp4", tag="t")
                nc.tensor.matmul(qp[:], wq_bf[:], xq_bf[:, sl],
                                 start=True, stop=True)
                nc.vector.tensor_scalar_add(q_f32[:, sl], qp[:], bq_sb[:])
                nc.vector.tensor_copy(q_bf[:, sl], q_f32[:, sl])

            q_done4 = 4

            vt = spool.tile([P, NTK, VT_STRIDE], fp8, name="vt")

            def emit_s_exp(chunk, blk, e_sw, dve=False):
                """Scores for one tk block + exp into the DRSW pair tile.

                e_sw is [P, 8, 256]: e_sw[p, j, 2*(127-m)+r] =
                exp(S^T[tk=blk, tq=chunk*TQC + j*128 + m]), r = blk & 1.

                dve=True computes exp on the Vector engine instead via the
                Schraudolph int8 bit trick: round(8*log2(e)*s*scale + 55.656)
                written as int8 IS the fp8e4m3 bit pattern of ~exp(s*scale)
                (2-3% jitter, cancels largely under softmax normalization).
                """
                s_ps = pss.tile([P, TQC], f32, name="s_ps", tag="ps")
                for h in range(TQC // TQ):
                    nc.tensor.matmul(
                        s_ps[:, h * TQ:(h + 1) * TQ],
                        k_bf[:, blk * P:(blk + 1) * P],
                        q_bf[:, chunk * TQC + h * TQ:
                             chunk * TQC + (h + 1) * TQ],
                        start=True, stop=True)
                r = blk & 1
                if dve:
                    nc.vector.tensor_scalar(
                        e_sw[:][:, :, 254 + r::-2].bitcast(i8),
                        s_ps[:].rearrange("p (j n) -> p j n", j=8),
                        SCH_A, SCH_B,
                        op0=mybir.AluOpType.mult, op1=mybir.AluOpType.add)
                else:
                    nc.scalar.activation(
                        e_sw[:][:, :, 254 + r::-2],
                        s_ps[:].rearrange("p (j n) -> p j n", j=8),
                        AF.Exp, scale=inv_sqrt_hw)

            def emit_pv(o_tiles, e_sw, d):
                # d==0, j%OPACK==0 clears the whole bank (start=True) before
                # the other j's of that bank accumulate (PE runs in order).
                for j in range(8):
                    nc.tensor.matmul(o_tiles[j // OPACK][:, j % OPACK, 0:129],
                                     e_sw[:, j, :],
                                     vt[:, 2 * d:2 * d + 2, 0:129],
                                     start=(d == 0 and j % OPACK == 0),
                                     stop=(d == ND - 1),
                                     skip_group_check=True, perf_mode=DRSW)

            def emit_finalize_pass1(chunk, o_tiles):
                recs = []
                for t in range(len(o_tiles)):
                    rec = fpool.tile([P, OPACK], f32, name="rec", tag="rec",
                                     bufs=4)
                    nc.vector.reciprocal(rec[:], o_tiles[t][:, :, 128])
                    recs.append(rec)
                an_tiles = []
                for j in range(8):
                    o_ap = o_tiles[j // OPACK][:, j % OPACK, :]
                    an = fpool.tile([P, P], bf16, name="an", tag="an", bufs=8)
                    nc.vector.tensor_scalar_mul(
                        an[:], o_ap[:, 0:128],
                        recs[j // OPACK][:, j % OPACK:j % OPACK + 1])
                    an_tiles.append(an)
                return an_tiles

            def emit_finalize_pass2(chunk, an_tiles, j):
                tq0 = chunk * TQC + j * P
                tp2 = pst.tile([P, P], bf16, name="tp2", tag="t")
                nc.tensor.transpose(tp2[:], an_tiles[j][:], ident_b[:])
                ob = fpool.tile([P, P], f32, name="ob", tag="ob", bufs=4)
                nc.vector.tensor_add(ob[:], tp2[:],
                                     q_f32[:, tq0:tq0 + P])
                nc.sync.dma_start(out[:, tq0:tq0 + P], ob[:])

            def alloc_o_tiles():
                ngroups = (8 + OPACK - 1) // OPACK
                return [
                    pso.tile([P, OPACK, 129], f32, name="o_ps", tag="o")
                    for _ in range(ngroups)
                ]

            # ---- chunk 0 head interleaved with the V transposes ----
            nc.gpsimd.memset(vt[:, :, 128:129], 1.0)

            def new_esw():
                return epool.tile([P, 8, 256], fp8, name="e_sw", tag="esw")

            with tc.tile_pool(name="ps_vt", bufs=3, space="PSUM") as pvt:
                pre0 = []
                t0 = new_esw()
                emit_s_exp(0, 0, t0)
                emit_s_exp(0, 1, t0)
                pre0.append(t0)
                for blk in range(NTK):
                    tp = pvt.tile([P, P], f32, name="vtp", tag="vtp")
                    nc.tensor.transpose(tp[:], xv_sb[:, blk * P:(blk + 1) * P],
                                        ident_f[:])
                    nc.vector.tensor_copy(vt[:, blk, 0:128], tp[:])
                    if blk % 4 == 3:
                        if k_done < HW // TQ:
                            k_proj(k_done, pst, "t")
                            k_done += 1
                        if len(pre0) < 4:
                            t = new_esw()
                            emit_s_exp(0, 2 * len(pre0), t)
                            emit_s_exp(0, 2 * len(pre0) + 1, t)
                            pre0.append(t)

            pso = tc.alloc_tile_pool(name="ps_o", bufs=OPACK, space="PSUM")

            pending = None   # (chunk, o_tiles) awaiting pass1
            deferred = None  # (chunk, an_tiles) awaiting pass2 units
            for chunk in range(NCHUNK):
                nprefd = 4 if chunk == 0 else PREFD
                if chunk == 0:
                    pre = pre0
                else:
                    pre = []
                    for d in range(nprefd):
                        t = new_esw()
                        emit_s_exp(chunk, 2 * d, t, dve=(d % 3 == 2))
                        emit_s_exp(chunk, 2 * d + 1, t, dve=(d % 3 == 2))
                        pre.append(t)
                if pending is not None:
                    deferred = (pending[0], emit_finalize_pass1(*pending))
                    pending = None
                o_tiles = alloc_o_tiles()
                for d in range(nprefd):
                    emit_pv(o_tiles, pre[d], d)
                p2 = 0
                for d in range(nprefd, ND):
                    if deferred is not None and p2 < 8:
                        emit_finalize_pass2(deferred[0], deferred[1], p2)
                        p2 += 1
                        if p2 == 8:
                            deferred = None
                    if d in (5, 7, 9, 11) and \
                            q_done4 < min(4 * (chunk + 2), 4 * NCHUNK):
                        q_proj256(q_done4)
                        q_done4 += 1
                    e_sw = new_esw()
                    emit_s_exp(chunk, 2 * d, e_sw, dve=(d % 3 == 2))
                    emit_s_exp(chunk, 2 * d + 1, e_sw, dve=(d % 3 == 2))
                    emit_pv(o_tiles, e_sw, d)
                pending = (chunk, o_tiles)
            an_last = emit_finalize_pass1(*pending)
            for j in range(8):
                emit_finalize_pass2(NCHUNK - 1, an_last, j)
            pso.release()
            pst.release()

    nc.finalize()
    return nc


def kernel(query_img, key_img, value_img, Wq, bq, Wk, bk):
    from concourse.bass_utils import run_bass_kernel_spmd

    global LAST_RESULTS

    query_img = np.asarray(query_img, dtype=np.float32)
    key_img = np.asarray(key_img, dtype=np.float32)
    value_img = np.asarray(value_img, dtype=np.float32)
    wqT = np.ascontiguousarray(np.asarray(Wq, dtype=np.float32).T)
    wkT = np.ascontiguousarray(np.asarray(Wk, dtype=np.float32).T)
    bqc = np.ascontiguousarray(np.asarray(bq, dtype=np.float32).reshape(C, 1))
    bkc = np.ascontiguousarray(np.asarray(bk, dtype=np.float32).reshape(C, 1))

    if "nc" not in _CACHE:
        _CACHE["nc"] = _build_kernel()
    nc = _CACHE["nc"]

    in_maps = []
    for b in range(B):
        in_maps.append({
            "xq": np.ascontiguousarray(query_img[b].reshape(C, HW)),
            "xk": np.ascontiguousarray(key_img[b].reshape(C, HW)),
            "xv": np.ascontiguousarray(value_img[b].reshape(C, HW)),
            "wqT": wqT,
            "wkT": wkT,
            "bqv": bqc,
            "bkv": bkc,
        })

    trace = os.environ.get("KERNEL_TRACE", "0") == "1"
    res = run_bass_kernel_spmd(nc, in_maps, core_ids=list(range(B)),
                               trace=trace)
    LAST_RESULTS = res
    out = np.stack([res.results[b]["out"].reshape(C, H, W) for b in range(B)])
    return out.astype(np.float32)

